# revision 20
# baseline (speedup 1.0000x reference)
"""MoD wrapper (router -> top-k -> gather -> GELU MLP -> weighted scatter-add)
on 8 Trainium2 NeuronCores.

Sharding: data-parallel over batch (4 sequences) x 2-way split of each
sequence's K=2048 selected tokens -> 8 cores, 1024 tokens each. Every core
holds the full FFN weights (fp8 e4m3) and computes
    y[t, :] = gate[t] * gelu_tanh(x[t, :] @ w1 + b1) @ w2
for its 1024 tokens. Both matmuls run in fp8 DoubleRow mode (2 fp8 MACs per
PE cell per cycle). Host-side power-of-2 scaling keeps the quantized values
inside e4m3's normal range: x*16, w1*64, w2*128; the descales are folded
into the gelu activation's scale argument (1/1024) and the host-side gate
values (gate/128), so no extra device ops are needed. The intermediate
h = gelu(x@w1+b1) is stored in SBUF as fp8 at natural scale (std ~0.6,
comfortably inside e4m3 range). Routing (scores / top-k / sigmoid) runs
through the same jax ops the reference uses, so token selection matches the
reference bit-for-bit; gather and the final scatter-add into the residual
stream are host-side numpy (b2 is folded into the scatter).
"""

import sys
import types

import numpy as np
import ml_dtypes

# bass_utils' trace path does `from antenv.axon_hooks import ...`; some
# images ship an antenv without that module (boot degrades silently but the
# import in bass_utils would crash). Register a no-op stand-in so trace=True
# degrades to "no profile" instead of raising.
try:
    import antenv.axon_hooks  # noqa: F401
except Exception:
    import antenv

    _hooks = types.ModuleType("antenv.axon_hooks")
    _hooks._hook = None
    _hooks.set_axon_ntff_profile_hook = \
        lambda h: setattr(_hooks, "_hook", h)
    _hooks.get_axon_ntff_profile_hook = \
        lambda: getattr(_hooks, "_hook", None)
    sys.modules["antenv.axon_hooks"] = _hooks
    antenv.axon_hooks = _hooks
    try:
        # Same registration trn_agent_boot.boot() would have done had the
        # module existed at interpreter start.
        from trn_agent_boot.trn_boot import _ntff_profile_via_ctypes

        _hook = _ntff_profile_via_ctypes("/opt/axon/libaxon_pjrt.so")
        if _hook is not None:
            _hooks.set_axon_ntff_profile_hook(_hook)
    except Exception:
        pass

import concourse.bacc as bacc
import concourse.bass as bass
import concourse.mybir as mybir
import concourse.tile as tile
from concourse.bass import ts
from concourse.bass_utils import run_bass_kernel_spmd
from concourse.kernels.tile_matmul import (
    ShapeInfo,
    composable_matmul_tile_kernel,
    dma_from_dram_kxm,
    dma_from_dram_kxn,
)

B, S, D, DFF = 4, 4096, 2048, 8192
K = 2048          # selected tokens per sequence
N_CORES = 8
TPC = (B * K) // N_CORES  # tokens per core = 1024

FP8 = mybir.dt.float8e4
FP8_NP = ml_dtypes.float8_e4m3  # TRN variant: max normal 240
F32 = mybir.dt.float32
BF16 = mybir.dt.bfloat16
P = 128

SX = 16.0    # x quant scale
SW1 = 64.0   # w1 quant scale (w1 std 0.022 -> 1.4)
SW2 = 128.0  # w2 quant scale (w2 std 0.011 -> 1.4)


def _build_nc():
    nc = bacc.Bacc("TRN2", target_bir_lowering=False, debug=False,
                   num_devices=N_CORES)

    xT_ap = nc.dram_tensor("xT", [D, TPC], FP8, kind="ExternalInput").ap()
    w1_ap = nc.dram_tensor("w1", [D, DFF], FP8, kind="ExternalInput").ap()
    w2_ap = nc.dram_tensor("w2", [DFF, D], FP8, kind="ExternalInput").ap()
    gate_ap = nc.dram_tensor("gate", [P, TPC // P], F32, kind="ExternalInput").ap()
    b1_ap = nc.dram_tensor("b1v", [P, DFF // P], F32, kind="ExternalInput").ap()
    y_ap = nc.dram_tensor("y", [TPC, D], BF16, kind="ExternalOutput").ap()

    with tile.TileContext(nc) as tc:
        with (
            tc.tile_pool(name="const", bufs=1) as const_pool,
            tc.tile_pool(name="hT", bufs=1) as hT_pool,
            tc.tile_pool(name="kxm1", bufs=5) as kxm1_pool,
            tc.tile_pool(name="kxn1", bufs=5) as kxn1_pool,
            tc.tile_pool(name="kxn2", bufs=17) as kxn2_pool,
        ):
            gate_sb = const_pool.tile([P, TPC // P], F32)
            b1_sb = const_pool.tile([P, DFF // P], F32)
            nc.gpsimd.dma_start(gate_sb[:], gate_ap[:])
            nc.gpsimd.dma_start(b1_sb[:], b1_ap[:])

            # Intermediate hT[f, t] = gelu(w1.T @ x.T + b1), kept in SBUF
            # as the kxm operand of the second matmul. [128, 64, 1024] fp8.
            hT_cache = hT_pool.tile([P, DFF // P, TPC], FP8)

            # ---- phase 1: hT = gelu((w1*SW1).T @ (xT*SX) / (SX*SW1) + b1) ----
            kxm1_producer, kxm1_shape = dma_from_dram_kxm(kxm1_pool, w1_ap)
            kxn1_producer, kxn1_shape = dma_from_dram_kxn(kxn1_pool, xT_ap)

            def hT_slice_producer(nc_, md):
                return hT_cache[:, ts(md.m_tile_idx, md.m_subtiles), md.n_slice]

            def gelu_reducer(nc_, psum, sbuf, md):
                f_outer = md.m_tile_idx * md.m_subtiles + md.m_subtile_idx
                nc_.scalar.activation(
                    sbuf,
                    psum,
                    mybir.ActivationFunctionType.Gelu_apprx_tanh,
                    bias=b1_sb[:, f_outer:f_outer + 1],
                    scale=1.0 / (SX * SW1),
                )

            composable_matmul_tile_kernel(
                tc,
                kxm_shape=kxm1_shape,
                kxn_shape=kxn1_shape,
                output_type=None,
                kxm_producer=kxm1_producer,
                kxn_producer=kxn1_producer,
                mxn_consumer=lambda nc_, t, md: None,
                mxn_subtile_reducer=gelu_reducer,
                mxn_subtile_producer=hT_slice_producer,
                cache_tiles=True,
            )

            # ---- phase 2: y = (hT.T @ (w2*SW2)) * (gate/SW2) ----
            kxm2_shape = ShapeInfo(pdims=((P, DFF // P),), fdims=(TPC,))

            def hT_kxm_producer(nc_, md):
                return hT_cache[:, ts(md.k_tile_idx, md.k_subtiles),
                                ts(md.m_tile_idx, md.m_tile)]

            kxn2_producer, kxn2_shape = dma_from_dram_kxn(kxn2_pool, w2_ap)

            def gate_reducer(nc_, psum, sbuf, md):
                t_outer = md.m_tile_idx * md.m_subtiles + md.m_subtile_idx
                nc_.vector.tensor_scalar_mul(
                    sbuf, psum, gate_sb[:, t_outer:t_outer + 1])
                # DMA each [128, 512] subtile out as soon as it drains,
                # instead of one 1MB burst per block: keeps the output
                # traffic off the critical path of the w2 prefetch.
                row0 = md.m_tile_idx * md.m_tile + md.m_subtile_idx * P
                col0 = md.n_tile_idx * md.n_tile \
                    + md.n_subtile_idx * md.n_subtile
                ncols = min(md.n_subtile, md.n_slice_size)
                nc_.sync.dma_start(
                    y_ap[row0:row0 + P, col0:col0 + ncols],
                    sbuf[:, 0, :ncols])

            composable_matmul_tile_kernel(
                tc,
                kxm_shape=kxm2_shape,
                kxn_shape=kxn2_shape,
                output_type=BF16,
                kxm_producer=hT_kxm_producer,
                kxn_producer=kxn2_producer,
                mxn_consumer=lambda nc_, t, md: None,
                mxn_subtile_reducer=gate_reducer,
                cache_tiles=True,
                psum_n_bufs=2,
            )

    nc.compile()
    return nc


_NC = None


def _routing(hidden_states, router_weight, router_bias):
    """Same ops/backend as the reference => bit-identical selection."""
    import jax
    import jax.numpy as jnp
    scores = jnp.einsum('bsd,d->bs', hidden_states, router_weight) \
        + router_bias[0]
    top_scores, indices = jax.lax.top_k(scores, K)
    weights = jax.nn.sigmoid(top_scores)
    return np.asarray(indices), np.asarray(weights)


def _q8(a, scale):
    return np.clip(a * scale, -240.0, 240.0).astype(FP8_NP)


def _run(hidden_states, router_weight, router_bias, w1, b1, w2, b2,
         trace=False):
    global _NC
    hidden_states = np.asarray(hidden_states, dtype=np.float32)
    router_weight = np.asarray(router_weight, dtype=np.float32)
    router_bias = np.asarray(router_bias, dtype=np.float32)
    w1 = np.asarray(w1, dtype=np.float32)
    b1 = np.asarray(b1, dtype=np.float32)
    w2 = np.asarray(w2, dtype=np.float32)
    b2 = np.asarray(b2, dtype=np.float32)

    indices, weights = _routing(hidden_states, router_weight, router_bias)

    if _NC is None:
        _NC = _build_nc()

    w1_q = _q8(w1, SW1)
    w2_q = _q8(w2, SW2)
    b1v = np.ascontiguousarray(b1.reshape(DFF // P, P).T)

    in_maps = []
    core_idx = []  # (b, idx_slice) per core
    for c in range(N_CORES):
        b, h = divmod(c, 2)
        idx_c = indices[b, h * TPC:(h + 1) * TPC]
        gate_c = weights[b, h * TPC:(h + 1) * TPC]
        xT = _q8(hidden_states[b, idx_c].T, SX)
        gate_eff = gate_c.astype(np.float32) / SW2
        in_maps.append({
            "xT": np.ascontiguousarray(xT),
            "w1": w1_q,
            "w2": w2_q,
            "gate": np.ascontiguousarray(gate_eff.reshape(TPC // P, P).T),
            "b1v": b1v,
        })
        core_idx.append((b, idx_c, gate_c))

    res = run_bass_kernel_spmd(_NC, in_maps, core_ids=list(range(N_CORES)),
                               trace=trace)

    out = hidden_states.copy().reshape(B * S, D)
    b2_nonzero = bool(np.any(b2))
    for c in range(N_CORES):
        b, idx_c, gate_c = core_idx[c]
        y = np.asarray(res.results[c]["y"]).astype(np.float32)
        if b2_nonzero:
            y = y + gate_c[:, None] * b2[None, :]
        out[b * S + idx_c] += y
    return out.reshape(B, S, D), res


def kernel(**inputs):
    return _run(**inputs)[0]


# revision 24
# speedup vs baseline: 1.0175x; 1.0175x over previous
"""MoD wrapper (router -> top-k -> gather -> GELU MLP -> weighted scatter-add)
on 8 Trainium2 NeuronCores.

Sharding: data-parallel over batch (4 sequences) x 2-way split of each
sequence's K=2048 selected tokens -> 8 cores, 1024 tokens each. Every core
holds the full FFN weights (fp8 e4m3) and computes
    y[t, :] = gate[t] * gelu_tanh(x[t, :] @ w1 + b1) @ w2
for its 1024 tokens. Both matmuls run in fp8 DoubleRow mode (2 fp8 MACs per
PE cell per cycle). Host-side power-of-2 scaling keeps the quantized values
inside e4m3's normal range: x*16, w1*64, w2*128; the descales are folded
into the gelu activation's scale argument (1/1024) and the host-side gate
values (gate/128), so no extra device ops are needed. The intermediate
h = gelu(x@w1+b1) is stored in SBUF as fp8 at natural scale (std ~0.6,
comfortably inside e4m3 range). Routing (scores / top-k / sigmoid) runs
through the same jax ops the reference uses, so token selection matches the
reference bit-for-bit; gather and the final scatter-add into the residual
stream are host-side numpy (b2 is folded into the scatter).
"""

import sys
import types

import numpy as np
import ml_dtypes

# bass_utils' trace path does `from antenv.axon_hooks import ...`; some
# images ship an antenv without that module (boot degrades silently but the
# import in bass_utils would crash). Register a no-op stand-in so trace=True
# degrades to "no profile" instead of raising.
try:
    import antenv.axon_hooks  # noqa: F401
except Exception:
    import antenv

    _hooks = types.ModuleType("antenv.axon_hooks")
    _hooks._hook = None
    _hooks.set_axon_ntff_profile_hook = \
        lambda h: setattr(_hooks, "_hook", h)
    _hooks.get_axon_ntff_profile_hook = \
        lambda: getattr(_hooks, "_hook", None)
    sys.modules["antenv.axon_hooks"] = _hooks
    antenv.axon_hooks = _hooks
    try:
        # Same registration trn_agent_boot.boot() would have done had the
        # module existed at interpreter start.
        from trn_agent_boot.trn_boot import _ntff_profile_via_ctypes

        _hook = _ntff_profile_via_ctypes("/opt/axon/libaxon_pjrt.so")
        if _hook is not None:
            _hooks.set_axon_ntff_profile_hook(_hook)
    except Exception:
        pass

import concourse.bacc as bacc
import concourse.bass as bass
import concourse.mybir as mybir
import concourse.tile as tile
from concourse.bass import ts
from concourse.bass_utils import run_bass_kernel_spmd
from concourse.kernels.tile_matmul import (
    ShapeInfo,
    composable_matmul_tile_kernel,
    dma_from_dram_kxm,
    dma_from_dram_kxn,
)

B, S, D, DFF = 4, 4096, 2048, 8192
K = 2048          # selected tokens per sequence
N_CORES = 8
TPC = (B * K) // N_CORES  # tokens per core = 1024

FP8 = mybir.dt.float8e4
FP8_NP = ml_dtypes.float8_e4m3  # TRN variant: max normal 240
F32 = mybir.dt.float32
BF16 = mybir.dt.bfloat16
P = 128

SX = 16.0    # x quant scale
SW1 = 64.0   # w1 quant scale (w1 std 0.022 -> 1.4)
SW2 = 128.0  # w2 quant scale (w2 std 0.011 -> 1.4)


def _build_nc(warmup=6, psum1_bufs=2):
    nc = bacc.Bacc("TRN2", target_bir_lowering=False, debug=False,
                   num_devices=N_CORES)

    xT_ap = nc.dram_tensor("xT", [D, TPC], FP8, kind="ExternalInput").ap()
    w1_ap = nc.dram_tensor("w1", [D, DFF], FP8, kind="ExternalInput").ap()
    w2_ap = nc.dram_tensor("w2", [DFF, D], FP8, kind="ExternalInput").ap()
    gate_ap = nc.dram_tensor("gate", [P, TPC // P], F32, kind="ExternalInput").ap()
    b1_ap = nc.dram_tensor("b1v", [P, DFF // P], F32, kind="ExternalInput").ap()
    y_ap = nc.dram_tensor("y", [TPC, D], BF16, kind="ExternalOutput").ap()

    with tile.TileContext(nc) as tc:
        with (
            tc.tile_pool(name="const", bufs=1) as const_pool,
            tc.tile_pool(name="hT", bufs=1) as hT_pool,
            tc.tile_pool(name="kxm1", bufs=5) as kxm1_pool,
            tc.tile_pool(name="kxn1", bufs=5) as kxn1_pool,
            tc.tile_pool(name="kxn2", bufs=17) as kxn2_pool,
        ):
            gate_sb = const_pool.tile([P, TPC // P], F32)
            b1_sb = const_pool.tile([P, DFF // P], F32)
            nc.gpsimd.dma_start(gate_sb[:], gate_ap[:])
            nc.gpsimd.dma_start(b1_sb[:], b1_ap[:])

            if warmup:
                # PE warm-up: dummy matmuls on a zero tile while the first
                # real weight tiles are in flight, so the first real matmuls
                # start at full clock instead of the cold-ramp rate.
                warm_sb = const_pool.tile([P, 512], BF16)
                nc.any.memset(warm_sb[:], 0.0)
                with tc.tile_pool(name="warm", bufs=1, space="PSUM") as wpool:
                    warm_ps = wpool.tile([P, 512], F32)
                    for _ in range(warmup):
                        nc.tensor.matmul(warm_ps[:], warm_sb[:, :P],
                                         warm_sb[:], start=True, stop=True)

            # Intermediate hT[f, t] = gelu(w1.T @ x.T + b1), kept in SBUF
            # as the kxm operand of the second matmul. [128, 64, 1024] fp8.
            hT_cache = hT_pool.tile([P, DFF // P, TPC], FP8)

            # ---- phase 1: hT = gelu((w1*SW1).T @ (xT*SX) / (SX*SW1) + b1) ----
            kxm1_producer, kxm1_shape = dma_from_dram_kxm(kxm1_pool, w1_ap)
            kxn1_producer, kxn1_shape = dma_from_dram_kxn(kxn1_pool, xT_ap)

            def hT_slice_producer(nc_, md):
                return hT_cache[:, ts(md.m_tile_idx, md.m_subtiles), md.n_slice]

            def gelu_reducer(nc_, psum, sbuf, md):
                f_outer = md.m_tile_idx * md.m_subtiles + md.m_subtile_idx
                nc_.scalar.activation(
                    sbuf,
                    psum,
                    mybir.ActivationFunctionType.Gelu_apprx_tanh,
                    bias=b1_sb[:, f_outer:f_outer + 1],
                    scale=1.0 / (SX * SW1),
                )

            composable_matmul_tile_kernel(
                tc,
                kxm_shape=kxm1_shape,
                kxn_shape=kxn1_shape,
                output_type=None,
                kxm_producer=kxm1_producer,
                kxn_producer=kxn1_producer,
                mxn_consumer=lambda nc_, t, md: None,
                mxn_subtile_reducer=gelu_reducer,
                mxn_subtile_producer=hT_slice_producer,
                cache_tiles=True,
                psum_n_bufs=psum1_bufs,
            )

            # ---- phase 2: y = (hT.T @ (w2*SW2)) * (gate/SW2) ----
            kxm2_shape = ShapeInfo(pdims=((P, DFF // P),), fdims=(TPC,))

            def hT_kxm_producer(nc_, md):
                return hT_cache[:, ts(md.k_tile_idx, md.k_subtiles),
                                ts(md.m_tile_idx, md.m_tile)]

            kxn2_producer, kxn2_shape = dma_from_dram_kxn(kxn2_pool, w2_ap)

            def gate_reducer(nc_, psum, sbuf, md):
                t_outer = md.m_tile_idx * md.m_subtiles + md.m_subtile_idx
                nc_.vector.tensor_scalar_mul(
                    sbuf, psum, gate_sb[:, t_outer:t_outer + 1])
                # DMA each [128, 512] subtile out as soon as it drains,
                # instead of one 1MB burst per block: keeps the output
                # traffic off the critical path of the w2 prefetch.
                row0 = md.m_tile_idx * md.m_tile + md.m_subtile_idx * P
                col0 = md.n_tile_idx * md.n_tile \
                    + md.n_subtile_idx * md.n_subtile
                ncols = min(md.n_subtile, md.n_slice_size)
                nc_.sync.dma_start(
                    y_ap[row0:row0 + P, col0:col0 + ncols],
                    sbuf[:, 0, :ncols])

            composable_matmul_tile_kernel(
                tc,
                kxm_shape=kxm2_shape,
                kxn_shape=kxn2_shape,
                output_type=BF16,
                kxm_producer=hT_kxm_producer,
                kxn_producer=kxn2_producer,
                mxn_consumer=lambda nc_, t, md: None,
                mxn_subtile_reducer=gate_reducer,
                cache_tiles=True,
                psum_n_bufs=2,
            )

    nc.compile()
    return nc


_NC = None


def _routing(hidden_states, router_weight, router_bias):
    """Same ops/backend as the reference => bit-identical selection."""
    import jax
    import jax.numpy as jnp
    scores = jnp.einsum('bsd,d->bs', hidden_states, router_weight) \
        + router_bias[0]
    top_scores, indices = jax.lax.top_k(scores, K)
    weights = jax.nn.sigmoid(top_scores)
    return np.asarray(indices), np.asarray(weights)


def _q8(a, scale):
    return np.clip(a * scale, -240.0, 240.0).astype(FP8_NP)


def _run(hidden_states, router_weight, router_bias, w1, b1, w2, b2,
         trace=False):
    global _NC
    hidden_states = np.asarray(hidden_states, dtype=np.float32)
    router_weight = np.asarray(router_weight, dtype=np.float32)
    router_bias = np.asarray(router_bias, dtype=np.float32)
    w1 = np.asarray(w1, dtype=np.float32)
    b1 = np.asarray(b1, dtype=np.float32)
    w2 = np.asarray(w2, dtype=np.float32)
    b2 = np.asarray(b2, dtype=np.float32)

    indices, weights = _routing(hidden_states, router_weight, router_bias)

    if _NC is None:
        _NC = _build_nc()

    w1_q = _q8(w1, SW1)
    w2_q = _q8(w2, SW2)
    b1v = np.ascontiguousarray(b1.reshape(DFF // P, P).T)

    in_maps = []
    core_idx = []  # (b, idx_slice) per core
    for c in range(N_CORES):
        b, h = divmod(c, 2)
        idx_c = indices[b, h * TPC:(h + 1) * TPC]
        gate_c = weights[b, h * TPC:(h + 1) * TPC]
        xT = _q8(hidden_states[b, idx_c].T, SX)
        gate_eff = gate_c.astype(np.float32) / SW2
        in_maps.append({
            "xT": np.ascontiguousarray(xT),
            "w1": w1_q,
            "w2": w2_q,
            "gate": np.ascontiguousarray(gate_eff.reshape(TPC // P, P).T),
            "b1v": b1v,
        })
        core_idx.append((b, idx_c, gate_c))

    res = run_bass_kernel_spmd(_NC, in_maps, core_ids=list(range(N_CORES)),
                               trace=trace)

    out = hidden_states.copy().reshape(B * S, D)
    b2_nonzero = bool(np.any(b2))
    for c in range(N_CORES):
        b, idx_c, gate_c = core_idx[c]
        y = np.asarray(res.results[c]["y"]).astype(np.float32)
        if b2_nonzero:
            y = y + gate_c[:, None] * b2[None, :]
        out[b * S + idx_c] += y
    return out.reshape(B, S, D), res


def kernel(**inputs):
    return _run(**inputs)[0]


# revision 30
# speedup vs baseline: 1.0184x; 1.0009x over previous
"""MoD wrapper (router -> top-k -> gather -> GELU MLP -> weighted scatter-add)
on 8 Trainium2 NeuronCores.

Sharding: data-parallel over batch (4 sequences) x 2-way split of each
sequence's K=2048 selected tokens -> 8 cores, 1024 tokens each. Every core
holds the full FFN weights (fp8 e4m3) and computes
    y[t, :] = gate[t] * gelu_tanh(x[t, :] @ w1 + b1) @ w2
for its 1024 tokens. Both matmuls run in fp8 DoubleRow mode (2 fp8 MACs per
PE cell per cycle). Host-side power-of-2 scaling keeps the quantized values
inside e4m3's normal range: x*16, w1*64, w2*128; the descales are folded
into the gelu activation's scale argument (1/1024) and the host-side gate
values (gate/128), so no extra device ops are needed. The intermediate
h = gelu(x@w1+b1) is stored in SBUF as fp8 at natural scale (std ~0.6,
comfortably inside e4m3 range). Routing (scores / top-k / sigmoid) runs
through the same jax ops the reference uses, so token selection matches the
reference bit-for-bit; gather and the final scatter-add into the residual
stream are host-side numpy (b2 is folded into the scatter).
"""

import sys
import types

import numpy as np
import ml_dtypes

# bass_utils' trace path does `from antenv.axon_hooks import ...`; some
# images ship an antenv without that module (boot degrades silently but the
# import in bass_utils would crash). Register a no-op stand-in so trace=True
# degrades to "no profile" instead of raising.
try:
    import antenv.axon_hooks  # noqa: F401
except Exception:
    import antenv

    _hooks = types.ModuleType("antenv.axon_hooks")
    _hooks._hook = None
    _hooks.set_axon_ntff_profile_hook = \
        lambda h: setattr(_hooks, "_hook", h)
    _hooks.get_axon_ntff_profile_hook = \
        lambda: getattr(_hooks, "_hook", None)
    sys.modules["antenv.axon_hooks"] = _hooks
    antenv.axon_hooks = _hooks
    try:
        # Same registration trn_agent_boot.boot() would have done had the
        # module existed at interpreter start.
        from trn_agent_boot.trn_boot import _ntff_profile_via_ctypes

        _hook = _ntff_profile_via_ctypes("/opt/axon/libaxon_pjrt.so")
        if _hook is not None:
            _hooks.set_axon_ntff_profile_hook(_hook)
    except Exception:
        pass

import concourse.bacc as bacc
import concourse.bass as bass
import concourse.mybir as mybir
import concourse.tile as tile
from concourse.bass import ts
from concourse.bass_utils import run_bass_kernel_spmd
from concourse.kernels.tile_matmul import (
    ShapeInfo,
    composable_matmul_tile_kernel,
    dma_from_dram_kxm,
    dma_from_dram_kxn,
)

B, S, D, DFF = 4, 4096, 2048, 8192
K = 2048          # selected tokens per sequence
N_CORES = 8
TPC = (B * K) // N_CORES  # tokens per core = 1024

FP8 = mybir.dt.float8e4
FP8_NP = ml_dtypes.float8_e4m3  # TRN variant: max normal 240
F32 = mybir.dt.float32
BF16 = mybir.dt.bfloat16
P = 128

SX = 16.0    # x quant scale
SW1 = 64.0   # w1 quant scale (w1 std 0.022 -> 1.4)
SW2 = 128.0  # w2 quant scale (w2 std 0.011 -> 1.4)


def _build_nc(warmup=6, psum1_bufs=2):
    nc = bacc.Bacc("TRN2", target_bir_lowering=False, debug=False,
                   num_devices=N_CORES)

    xT_ap = nc.dram_tensor("xT", [D, TPC], FP8, kind="ExternalInput").ap()
    w1_ap = nc.dram_tensor("w1", [D, DFF], FP8, kind="ExternalInput").ap()
    w2_ap = nc.dram_tensor("w2", [DFF, D], FP8, kind="ExternalInput").ap()
    gate_ap = nc.dram_tensor("gate", [P, TPC // P], F32, kind="ExternalInput").ap()
    b1_ap = nc.dram_tensor("b1v", [P, DFF // P], F32, kind="ExternalInput").ap()
    y_ap = nc.dram_tensor("y", [TPC, D], BF16, kind="ExternalOutput").ap()

    with tile.TileContext(nc) as tc:
        with (
            tc.tile_pool(name="const", bufs=1) as const_pool,
            tc.tile_pool(name="hT", bufs=1) as hT_pool,
            tc.tile_pool(name="kxm1", bufs=5) as kxm1_pool,
            tc.tile_pool(name="kxn1", bufs=5) as kxn1_pool,
            tc.tile_pool(name="kxn2", bufs=17) as kxn2_pool,
        ):
            gate_sb = const_pool.tile([P, TPC // P], F32)
            b1_sb = const_pool.tile([P, DFF // P], F32)
            nc.gpsimd.dma_start(gate_sb[:], gate_ap[:])
            nc.gpsimd.dma_start(b1_sb[:], b1_ap[:])

            if warmup:
                # PE warm-up: dummy matmuls on a zero tile while the first
                # real weight tiles are in flight, so the first real matmuls
                # start at full clock instead of the cold-ramp rate.
                warm_sb = const_pool.tile([P, 512], BF16)
                nc.any.memset(warm_sb[:], 0.0)
                with tc.tile_pool(name="warm", bufs=1, space="PSUM") as wpool:
                    warm_ps = wpool.tile([P, 512], F32)
                    for _ in range(warmup):
                        nc.tensor.matmul(warm_ps[:], warm_sb[:, :P],
                                         warm_sb[:], start=True, stop=True)

            # Intermediate hT[f, t] = gelu(w1.T @ x.T + b1), kept in SBUF
            # as the kxm operand of the second matmul. [128, 64, 1024] fp8.
            hT_cache = hT_pool.tile([P, DFF // P, TPC], FP8)

            # ---- phase 1: hT = gelu((w1*SW1).T @ (xT*SX) / (SX*SW1) + b1) ----
            kxm1_producer, kxm1_shape = dma_from_dram_kxm(kxm1_pool, w1_ap)
            kxn1_producer, kxn1_shape = dma_from_dram_kxn(kxn1_pool, xT_ap)

            def hT_slice_producer(nc_, md):
                return hT_cache[:, ts(md.m_tile_idx, md.m_subtiles), md.n_slice]

            def gelu_reducer(nc_, psum, sbuf, md):
                f_outer = md.m_tile_idx * md.m_subtiles + md.m_subtile_idx
                nc_.scalar.activation(
                    sbuf,
                    psum,
                    mybir.ActivationFunctionType.Gelu_apprx_tanh,
                    bias=b1_sb[:, f_outer:f_outer + 1],
                    scale=1.0 / (SX * SW1),
                )

            composable_matmul_tile_kernel(
                tc,
                kxm_shape=kxm1_shape,
                kxn_shape=kxn1_shape,
                output_type=None,
                kxm_producer=kxm1_producer,
                kxn_producer=kxn1_producer,
                mxn_consumer=lambda nc_, t, md: None,
                mxn_subtile_reducer=gelu_reducer,
                mxn_subtile_producer=hT_slice_producer,
                cache_tiles=True,
                psum_n_bufs=psum1_bufs,
            )

            # ---- phase 2: y = (hT.T @ (w2*SW2)) * (gate/SW2) ----
            kxm2_shape = ShapeInfo(pdims=((P, DFF // P),), fdims=(TPC,))

            def hT_kxm_producer(nc_, md):
                return hT_cache[:, ts(md.k_tile_idx, md.k_subtiles),
                                ts(md.m_tile_idx, md.m_tile)]

            kxn2_producer, kxn2_shape = dma_from_dram_kxn(kxn2_pool, w2_ap)

            def gate_reducer(nc_, psum, sbuf, md):
                t_outer = md.m_tile_idx * md.m_subtiles + md.m_subtile_idx
                nc_.vector.tensor_scalar_mul(
                    sbuf, psum, gate_sb[:, t_outer:t_outer + 1])
                # DMA each [128, 512] subtile out as soon as it drains,
                # instead of one 1MB burst per block: keeps the output
                # traffic off the critical path of the w2 prefetch.
                row0 = md.m_tile_idx * md.m_tile + md.m_subtile_idx * P
                col0 = md.n_tile_idx * md.n_tile \
                    + md.n_subtile_idx * md.n_subtile
                ncols = min(md.n_subtile, md.n_slice_size)
                nc_.sync.dma_start(
                    y_ap[row0:row0 + P, col0:col0 + ncols],
                    sbuf[:, 0, :ncols])

            composable_matmul_tile_kernel(
                tc,
                kxm_shape=kxm2_shape,
                kxn_shape=kxn2_shape,
                output_type=BF16,
                kxm_producer=hT_kxm_producer,
                kxn_producer=kxn2_producer,
                mxn_consumer=lambda nc_, t, md: None,
                mxn_subtile_reducer=gate_reducer,
                cache_tiles=True,
                psum_n_bufs=2,
            )

    nc.compile()
    return nc


_NC = None


def _routing(hidden_states, router_weight, router_bias):
    """Same ops/backend as the reference => bit-identical selection."""
    import jax
    import jax.numpy as jnp
    scores = jnp.einsum('bsd,d->bs', hidden_states, router_weight) \
        + router_bias[0]
    top_scores, indices = jax.lax.top_k(scores, K)
    weights = jax.nn.sigmoid(top_scores)
    return np.asarray(indices), np.asarray(weights)


def _q8(a, scale):
    return np.clip(a * scale, -240.0, 240.0).astype(FP8_NP)


def _run(hidden_states, router_weight, router_bias, w1, b1, w2, b2,
         trace=False):
    global _NC
    hidden_states = np.asarray(hidden_states, dtype=np.float32)
    router_weight = np.asarray(router_weight, dtype=np.float32)
    router_bias = np.asarray(router_bias, dtype=np.float32)
    w1 = np.asarray(w1, dtype=np.float32)
    b1 = np.asarray(b1, dtype=np.float32)
    w2 = np.asarray(w2, dtype=np.float32)
    b2 = np.asarray(b2, dtype=np.float32)

    indices, weights = _routing(hidden_states, router_weight, router_bias)

    if _NC is None:
        _NC = _build_nc()

    w1_q = _q8(w1, SW1)
    w2_q = _q8(w2, SW2)
    b1v = np.ascontiguousarray(b1.reshape(DFF // P, P).T)

    in_maps = []
    core_idx = []  # (b, idx_slice) per core
    for c in range(N_CORES):
        b, h = divmod(c, 2)
        idx_c = indices[b, h * TPC:(h + 1) * TPC]
        gate_c = weights[b, h * TPC:(h + 1) * TPC]
        xT = _q8(hidden_states[b, idx_c].T, SX)
        gate_eff = gate_c.astype(np.float32) / SW2
        in_maps.append({
            "xT": np.ascontiguousarray(xT),
            "w1": w1_q,
            "w2": w2_q,
            "gate": np.ascontiguousarray(gate_eff.reshape(TPC // P, P).T),
            "b1v": b1v,
        })
        core_idx.append((b, idx_c, gate_c))

    res = run_bass_kernel_spmd(_NC, in_maps, core_ids=list(range(N_CORES)),
                               trace=trace)

    out = hidden_states.copy().reshape(B * S, D)
    b2_nonzero = bool(np.any(b2))
    for c in range(N_CORES):
        b, idx_c, gate_c = core_idx[c]
        y = np.asarray(res.results[c]["y"]).astype(np.float32)
        if b2_nonzero:
            y = y + gate_c[:, None] * b2[None, :]
        out[b * S + idx_c] += y
    return out.reshape(B, S, D), res


def kernel(**inputs):
    return _run(**inputs)[0]
